# revision 26
# baseline (speedup 1.0000x reference)
# Multi-head attention (B=2, S=2048, D=1024, H=16, d=64) on 8 NeuronCores.
#
# Sharding: core c handles batch b = c//4 and head group g = c%4 (4 heads).
# Column-shard qw/kw/vw by head group, row-shard ow; partial outputs are
# summed on the host per batch.
#
# Per-core dataflow (everything in transposed [feature, seq] layout):
#   Q^T = wq_g.T @ x^T   (fp32 matmul, wq pre-scaled by 1/sqrt(D) on host)
#   K^T = wk_g.T @ x^T   (fp32)
#   V   = x @ wv_g       (fp32, natural [seq, d] layout, + ones column)
#   RoPE on Q^T/K^T rows (cos/sin tables precomputed on host, sign-folded)
#   pass 1: S[q,k] tiles in bf16 -> row max m[q]  (only needs +-80 accuracy)
#   m column -> row via PE transpose with a negated identity, bounced
#     through DRAM into row 64 of the extended Q operand
#   pass 2: S^T[k,q] = [K^T;1].T @ [Q^T;-m]  (fp32, 65-dim contraction folds
#     the max subtraction into the matmul) -> exp on ACT -> P^T in SBUF
#   PV: [V|1].T @ P^T accumulated over k tiles (float32r, full PE rate);
#     row 64 gives the softmax denominator l[q] for free
#   merged^T = attn^T * (1/l) ;  out_partial = merged^T.T @ ow_g (float32r)

import numpy as np

_STATE = {}

B, S, D, H, HD = 2, 2048, 1024, 16, 64
HPC = 4          # heads per core
GC = HPC * HD    # columns per core = 256
NKC = D // 128   # k chunks for d_model contraction = 8
NST = S // 128   # seq tiles = 16
NSC = S // 512   # seq chunks of 512 = 4


def _build():
    import concourse.tile as tile
    import concourse.mybir as mybir
    from concourse import bacc
    from concourse.masks import make_identity

    f32 = mybir.dt.float32
    f32r = mybir.dt.float32r
    bf16 = mybir.dt.bfloat16

    nc = bacc.Bacc(None, target_bir_lowering=False, debug=False)

    xt_d = nc.dram_tensor("xt", [D, S], f32r, kind="ExternalInput")
    wq_d = nc.dram_tensor("wq", [D, GC], f32r, kind="ExternalInput")
    wk_d = nc.dram_tensor("wk", [D, GC], f32r, kind="ExternalInput")
    wv_d = nc.dram_tensor("wv", [D, GC], f32r, kind="ExternalInput")
    wo_d = nc.dram_tensor("wo", [GC, D], f32r, kind="ExternalInput")
    ones_d = nc.dram_tensor("onesr", [1, S], f32r, kind="ExternalInput")
    cos_d = nc.dram_tensor("cos64", [128, S], f32, kind="ExternalInput")
    sin_d = nc.dram_tensor("sin64", [128, S], f32, kind="ExternalInput")
    out_d = nc.dram_tensor("out", [S, D], f32, kind="ExternalOutput")
    dbg_d = nc.dram_tensor("dbg", [HPC, S], f32, kind="ExternalOutput")

    with tile.TileContext(nc) as tc:
        with (
            tc.tile_pool(name="sb", bufs=1) as sb,
            tc.tile_pool(name="dram", bufs=2, space="DRAM") as dram,
            tc.tile_pool(name="ps", bufs=2, space="PSUM") as psp,
        ):
            # ---- setup tiles ----
            v_sb = sb.tile([128, NST, HPC, HD + 1], bf16, tag="v", name="v_sb")
            onescol = sb.tile([128, 1], f32, tag="onescol", name="onescol")
            nc.vector.memset(onescol, 1.0)
            nc.vector.tensor_copy(
                out=v_sb[:, :, :, HD],
                in_=onescol.to_broadcast([128, NST, HPC]),
            )
            ident = sb.tile([128, 128], f32, tag="idn", name="ident")
            make_identity(nc, ident)

            # qk weights first (consumed immediately), then v, then the
            # tables (not needed until rope, ~40us in), wo last.
            wq_sb = sb.tile([128, NKC, GC], f32r, tag="wq", name="wq_sb")
            wk_sb = sb.tile([128, NKC, GC], f32r, tag="wk", name="wk_sb")
            wv_sb = sb.tile([128, NKC, GC], f32r, tag="wv", name="wv_sb")
            for kc in range(NKC):
                for w_sb, w_d in ((wq_sb, wq_d), (wk_sb, wk_d), (wv_sb, wv_d)):
                    nc.sync.dma_start(
                        out=w_sb[:, kc, :],
                        in_=w_d[kc * 128 : (kc + 1) * 128, :],
                    )
            cos_t = sb.tile([128, S], f32, tag="cos", name="cos_t")
            sin_t = sb.tile([128, S], f32, tag="sin", name="sin_t")
            nc.sync.dma_start(out=cos_t, in_=cos_d[:])
            nc.sync.dma_start(out=sin_t, in_=sin_d[:])
            wo_sb = sb.tile([128, 2, D], f32r, tag="wo", name="wo_sb")
            nc.sync.dma_start(
                out=wo_sb, in_=wo_d[:].rearrange("(c p) n -> p c n", p=128)
            )

            ones1 = sb.tile([1, HD], f32r, tag="ones1", name="ones1")
            nc.sync.dma_start(out=ones1, in_=ones_d[:, 0:HD])

            mrg = [None, None]
            for ch in range(2):
                mrg[ch] = sb.tile([128, S], f32r, tag="mrg", bufs=2,
                                  name=f"mrg{ch}")

            def project(grp):
                """Q^T/K^T for head pair `grp` (and V for all heads on grp 0)."""
                qpre = sb.tile([128, S], f32, tag="pre", bufs=2, name=f"qp{grp}")
                kpre = sb.tile([128, S], f32, tag="pre", bufs=2, name=f"kp{grp}")
                for sc in range(NSC):
                    xt_sc = []
                    for kc in range(NKC):
                        xx = sb.tile([128, 512], f32r, tag="xt", bufs=8,
                                     name=f"xt{grp}_{sc}_{kc}")
                        nc.sync.dma_start(
                            out=xx,
                            in_=xt_d[
                                kc * 128 : (kc + 1) * 128,
                                sc * 512 : (sc + 1) * 512,
                            ],
                        )
                        xt_sc.append(xx)
                    for w_sb, pre in ((wq_sb, qpre), (wk_sb, kpre)):
                        ps = psp.tile([128, 512], f32, tag="ps", bufs=2, name="ps_qk")
                        for kc in range(NKC):
                            nc.tensor.matmul(
                                ps,
                                lhsT=w_sb[:, kc, grp * 128 : (grp + 1) * 128],
                                rhs=xt_sc[kc],
                                start=(kc == 0),
                                stop=(kc == NKC - 1),
                            )
                        nc.scalar.copy(
                            out=pre[:, sc * 512 : (sc + 1) * 512], in_=ps
                        )
                    if grp == 0:
                        for st4 in range(4):
                            st = sc * 4 + st4
                            ps = psp.tile([128, 512], f32, tag="ps", bufs=2, name="ps_v")
                            psv = ps[:, :GC]
                            for kc in range(NKC):
                                nc.tensor.matmul(
                                    psv,
                                    lhsT=xt_sc[kc][:, st4 * 128 : (st4 + 1) * 128],
                                    rhs=wv_sb[:, kc, :],
                                    start=(kc == 0),
                                    stop=(kc == NKC - 1),
                                )
                            nc.vector.tensor_copy(
                                out=v_sb[:, st, :, 0:HD],
                                in_=psv.rearrange("p (h d) -> p h d", h=HPC),
                            )
                return qpre, kpre

            def rope_grp(grp, qpre, kpre):
                """RoPE both heads of grp at once: [128, S] stacked layout
                (even head rows 0-63, odd head rows 64-127)."""
                qs = sb.tile([128, S], f32r, tag="qs", bufs=2, name=f"qs{grp}")
                ks = sb.tile([128, S], f32r, tag="ks", bufs=2, name=f"ks{grp}")
                for pre, ext in ((qpre, qs), (kpre, ks)):
                    sw = sb.tile([128, S], f32, tag="sw", bufs=2,
                                 name=f"sw{grp}")
                    for b0 in (0, HD):
                        nc.sync.dma_start(
                            out=sw[b0 : b0 + 32, :],
                            in_=pre[b0 + 32 : b0 + 64, :],
                        )
                        nc.sync.dma_start(
                            out=sw[b0 + 32 : b0 + 64, :],
                            in_=pre[b0 : b0 + 32, :],
                        )
                    nc.vector.tensor_mul(out=ext, in0=pre, in1=cos_t)
                    nc.vector.tensor_mul(out=sw, in0=sw, in1=sin_t)
                    nc.vector.tensor_add(out=ext, in0=ext, in1=sw)
                return qs, ks

            def extract_h1(grp, qs, ks):
                """Copy odd head rows 64-127 down to partitions 0-63 of
                fresh [65, S] tiles (pass2 needs contiguous rows 0-64)."""
                qx1 = sb.tile([HD + 1, S], f32r, tag="qx1", bufs=2,
                              name=f"qx1_{grp}")
                kx1 = sb.tile([HD + 1, S], f32r, tag="kx1", bufs=2,
                              name=f"kx1_{grp}")
                nc.sync.dma_start(out=qx1[0:HD, :], in_=qs[HD:128, :])
                nc.sync.dma_start(out=kx1[0:HD, :], in_=ks[HD:128, :])
                nc.sync.dma_start(out=kx1[HD : HD + 1, :], in_=ones_d[:])
                return qx1, kx1

            def pass1_pair(grp, qs, ks, qx1):
                """Row-packed f32r scores for both heads -> row maxes ->
                -m rows; ones row into ks row 64 for the even head."""
                mp = [
                    sb.tile([128, NST, NSC // 2], f32, tag="mpart", bufs=4,
                            name=f"mp{grp}_{i}")
                    for i in (0, 1)
                ]
                HT = NST // 2
                for half in range(2):
                    for qt in range(half * HT, (half + 1) * HT):
                        for c2 in range(NSC // 2):
                            for i, b0 in ((0, 0), (1, HD)):
                                psw = psp.tile([128, 1024], f32, tag="psw",
                                               bufs=2, name="ps_s1")
                                for kh in range(2):
                                    k0 = c2 * 1024 + kh * 512
                                    nc.tensor.matmul(
                                        psw[:, kh * 512 : (kh + 1) * 512],
                                        lhsT=qs[b0 : b0 + HD,
                                                qt * 128 : (qt + 1) * 128],
                                        rhs=ks[b0 : b0 + HD, k0 : k0 + 512],
                                        start=True,
                                        stop=True,
                                    )
                                nc.vector.reduce_max(
                                    out=mp[i][:, qt, c2 : c2 + 1],
                                    in_=psw,
                                    axis=mybir.AxisListType.X,
                                )
                    # per-half max -> -m row chunk, so pass2's matching
                    # q-half can start before the other half's reduces
                    for i, qdst in ((0, qs), (1, qx1)):
                        mcol = sb.tile([128, HT], f32, tag="mcol", bufs=8,
                                       name=f"mc{grp}_{i}_{half}")
                        nc.vector.reduce_max(
                            out=mcol,
                            in_=mp[i][:, half * HT : (half + 1) * HT, :],
                            axis=mybir.AxisListType.X,
                        )
                        pst = psp.tile([HT, 128], f32, tag="ps", bufs=2,
                                       name=f"tp{grp}_{i}_{half}")
                        nc.tensor.transpose(pst, mcol, ident)
                        msb = sb.tile([HT, 128], f32r, tag="msb", bufs=8,
                                      name=f"ms{grp}_{i}_{half}")
                        nc.vector.tensor_scalar_mul(msb, pst, -1.0)
                        mrow_d = dram.tile([1, S // 2], f32r, tag="mrow",
                                           bufs=4,
                                           name=f"mrow{grp}_{i}_{half}")
                        nc.sync.dma_start(
                            out=mrow_d[:].rearrange(
                                "o (a b) -> (o a) b", a=HT
                            ),
                            in_=msb,
                        )
                        nc.sync.dma_start(
                            out=qdst[HD : HD + 1,
                                     half * (S // 2) : (half + 1) * (S // 2)],
                            in_=mrow_d[:],
                        )
                nc.sync.dma_start(out=ks[HD : HD + 1, :], in_=ones_d[:])

            def pass2(h, qx, kx):
                """fp32 S^T -> exp -> PV (float32r) -> normalized merged^T."""
                ch, offr = h // 2, 64 * (h % 2)
                for qh in range(2):
                    pv = [None, None]
                    for c in range(2):
                        pv[c] = psp.tile([128, 512], f32, tag="pv",
                                         name=f"pv{h}_{qh}_{c}")
                    for kt in range(NST):
                        pt = sb.tile([128, 1024], bf16, tag="pt", bufs=2,
                                     name=f"pt{h}_{qh}_{kt}")
                        psw = psp.tile([128, 1024], f32, tag="psw", bufs=2,
                                       name="ps_s2")
                        for c in range(2):
                            q0 = qh * 1024 + c * 512
                            nc.tensor.matmul(
                                psw[:, c * 512 : (c + 1) * 512],
                                lhsT=kx[0 : HD + 1, kt * 128 : (kt + 1) * 128],
                                rhs=qx[0 : HD + 1, q0 : q0 + 512],
                                start=True,
                                stop=True,
                            )
                        nc.scalar.activation(
                            out=pt,
                            in_=psw,
                            func=mybir.ActivationFunctionType.Exp,
                        )
                        for c in range(2):
                            nc.tensor.matmul(
                                pv[c][: HD + 1, :],
                                lhsT=v_sb[:, kt, h, :],
                                rhs=pt[:, c * 512 : (c + 1) * 512],
                                start=(kt == 0),
                                stop=(kt == NST - 1),
                            )
                    for c in range(2):
                        q0 = qh * 1024 + c * 512
                        lsb = sb.tile([1, 512], f32r, tag="lsb", bufs=2,
                                      name=f"ls{h}_{qh}_{c}")
                        nc.scalar.copy(out=lsb, in_=pv[c][HD : HD + 1, :])
                        # replicate l across 64 partitions: ones[1,64].T @ l[1,512]
                        lps = psp.tile([HD, 512], f32, tag="ps", bufs=2,
                                       name=f"lp{h}_{qh}_{c}")
                        nc.tensor.matmul(lps, lhsT=ones1,
                                         rhs=lsb,
                                         start=True, stop=True)
                        recb = sb.tile([HD, 512], f32, tag="recb", bufs=2,
                                       name=f"rb{h}_{qh}_{c}")
                        nc.vector.reciprocal_approx_fast(out=recb, in_=lps)
                        nc.vector.tensor_mul(
                            out=mrg[ch][offr : offr + HD, q0 : q0 + 512],
                            in0=pv[c][0:HD, :],
                            in1=recb,
                        )

            # ---- pipeline ----
            # project(1) is emitted before grp0's attention so its matmuls
            # sit ready in the PE queue while rope(0) runs on the DVE (the
            # per-engine queues are FIFO: emission order = execution order).
            qpre0, kpre0 = project(0)
            qs0, ks0 = rope_grp(0, qpre0, kpre0)
            qx1_0, kx1_0 = extract_h1(0, qs0, ks0)
            qpre1, kpre1 = project(1)
            pass1_pair(0, qs0, ks0, qx1_0)
            qs1, ks1 = rope_grp(1, qpre1, kpre1)
            qx1_1, kx1_1 = extract_h1(1, qs1, ks1)
            pass2(0, qs0, ks0)
            pass1_pair(1, qs1, ks1, qx1_1)
            pass2(1, qx1_0, kx1_0)
            pass2(2, qs1, ks1)
            pass2(3, qx1_1, kx1_1)

            # ---- output projection (float32r) ----
            for qt in range(NST):
                for n in range(2):
                    ps = psp.tile([128, 512], f32, tag="ps", bufs=2, name="ps_o")
                    for ch in range(2):
                        nc.tensor.matmul(
                            ps,
                            lhsT=mrg[ch][:, qt * 128 : (qt + 1) * 128],
                            rhs=wo_sb[:, ch, n * 512 : (n + 1) * 512].bitcast(f32r),
                            start=(ch == 0),
                            stop=(ch == 1),
                        )
                    obuf = sb.tile([128, 512], f32, tag="obuf", bufs=2,
                                   name=f"ob{qt}_{n}")
                    nc.scalar.copy(out=obuf, in_=ps)
                    nc.sync.dma_start(
                        out=out_d[qt * 128 : (qt + 1) * 128,
                                  n * 512 : (n + 1) * 512],
                        in_=obuf,
                    )

    nc.compile()
    return nc


def _tables():
    j = np.arange(0, HD, 2, dtype=np.float32)
    inv_freq = (
        np.float32(1.0) / (np.float32(10000.0) ** (j / np.float32(HD)))
    ).astype(np.float32)
    freqs = np.arange(S, dtype=np.float32)[:, None] * inv_freq[None, :]  # [S, 32]
    cos = np.cos(freqs).astype(np.float32).T  # [32, S]
    sin = np.sin(freqs).astype(np.float32).T
    cos128 = np.concatenate([cos, cos, cos, cos], axis=0)  # [128, S]
    sin128 = np.concatenate([-sin, sin, -sin, sin], axis=0)
    return np.ascontiguousarray(cos128), np.ascontiguousarray(sin128)


def kernel(x, qw, kw, vw, ow):
    from concourse.bass_utils import run_bass_kernel_spmd

    if "nc" not in _STATE:
        _STATE["nc"] = _build()
    nc = _STATE["nc"]

    x = np.asarray(x, dtype=np.float32)
    qw = np.asarray(qw, dtype=np.float32)
    kw = np.asarray(kw, dtype=np.float32)
    vw = np.asarray(vw, dtype=np.float32)
    ow = np.asarray(ow, dtype=np.float32)

    cos64, sin64 = _tables()
    scale = np.float32(1.0 / 32.0)  # 1/sqrt(D), exact power of two

    in_maps = []
    for c in range(8):
        b, g = c // 4, c % 4
        sl = slice(g * GC, (g + 1) * GC)
        in_maps.append(
            {
                "xt": np.ascontiguousarray(x[b].T),
                "wq": np.ascontiguousarray(qw[:, sl]) * scale,
                "wk": np.ascontiguousarray(kw[:, sl]),
                "wv": np.ascontiguousarray(vw[:, sl]),
                "wo": np.ascontiguousarray(ow[sl, :]),
                "onesr": np.ones((1, S), dtype=np.float32),
                "cos64": cos64,
                "sin64": sin64,
            }
        )

    res = run_bass_kernel_spmd(nc, in_maps, core_ids=list(range(8)))
    _STATE["last_res"] = res
    outs = [r["out"] for r in res.results]
    full = np.empty((B, S, D), dtype=np.float32)
    for b in range(B):
        full[b] = sum(o.astype(np.float64) for o in outs[4 * b : 4 * b + 4]).astype(
            np.float32
        )
    return full

